# revision 1
# baseline (speedup 1.0000x reference)
"""BBoxTransform Trainium kernel: two SPMD launches of raw-Bass elementwise
kernels + host-side reshuffles.

Launch 1 (core b <-> batch b): from boxes/deltas planes compute
  xlo, xhi, ylo, yhi, ca, sa, tx, ty            (8 planes of N)
Launch 2 (core j <-> slice of flat output index n' = b*N+n): combine
  out_x = ca*V0 - sa*V1 + tx*V2,  out_y = sa*V0 + ca*V1 + ty*V2
where V are 12 deinterleaved phase-planes of the C-row stack (the
reference's cat(axis=0).reshape(B,N,3,4) scramble is a pure
reinterpretation of that stack).  n' >= NR touch only ones-rows and
degenerate to (ca-sa+tx, sa+ca+ty) broadcast over 4 corners.

Engine split: DVE + GpSimd share the elementwise work; ACT does the
transcendentals (exp/ln on the natural_log_exp table set, then sin on the
trig set -- exactly two table loads).  sin/cos of alpha come from nested
half-angle identities so every ACT Sin argument is within [-pi, pi].
"""

import math
from contextlib import ExitStack

import numpy as np

import concourse.bass as bass
import concourse.mybir as mybir
from concourse.bass_utils import run_bass_kernel_spmd

DT = mybir.dt.float32
P = 128
B, N = 8, 250000

# ---- launch-1 geometry ----
F1 = 1956                       # free size of a full [128, F1] plane
NP1 = P * F1                    # padded plane length 250368 (N + 368)
NCH1 = 6                        # chunks
FC1 = F1 // NCH1

# ---- launch-2 geometry ----
NR = -(-64 * N // 12)           # 1333334: n' below this touch real C rows
NO = 8 * N - NR                 # ones-region size 666666
NRC = -(-NR // 8)               # 166667 real n' per core
NOC = -(-NO // 8)               # 83334 ones n' per core
F2 = 1304                       # 128*1304 = 166912 >= NRC
NR2 = P * F2
F2O = 652                       # 128*652 = 83456 >= NOC
NO2 = P * F2O
NCH2 = 4
FC2 = F2 // NCH2

LN_HALF = float(math.log(0.5))
PI = float(np.float32(math.pi))
HALF_PI = float(np.float32(math.pi / 2))

AF = mybir.ActivationFunctionType
OP = mybir.AluOpType


def _register_const(nc, value):
    t = nc.alloc_sbuf_tensor(f"const-user-{value}", [128, 1], DT)
    nc.gpsimd.memset(t.ap(), value)
    nc.const_aps.aps[(DT, value)] = t.ap()


def build_l1():
    nc = bass.Bass(detect_race_conditions=False)
    _register_const(nc, LN_HALF)
    nc.all_engine_barrier()
    bx = nc.declare_dram_parameter("bx", [5, NP1], DT, isOutput=False)
    dl = nc.declare_dram_parameter("dl", [5, NP1], DT, isOutput=False)
    out = nc.declare_dram_parameter("out", [8, NP1], DT, isOutput=True)

    def dchunk(t, i, c):
        return t[i].rearrange("(p f) -> p f", p=P)[:, c * FC1:(c + 1) * FC1]

    with ExitStack() as ctx:
        T = {}
        for name in ("b0", "b1", "b2", "b3", "b4", "d0", "d1", "d2", "d3",
                     "d4", "u0", "u1", "hwh", "hhh", "w", "h", "a1", "a2",
                     "a3", "rs", "sy"):
            T[name] = ctx.enter_context(nc.sbuf_tensor(name, [P, F1], DT))
        g0 = [ctx.enter_context(nc.semaphore(f"g0_{c}")) for c in range(NCH1)]
        g1 = [ctx.enter_context(nc.semaphore(f"g1_{c}")) for c in range(NCH1)]
        g2 = [ctx.enter_context(nc.semaphore(f"g2_{c}")) for c in range(NCH1)]
        sact = ctx.enter_context(nc.semaphore("sact"))
        sdve = ctx.enter_context(nc.semaphore("sdve"))
        sgp = ctx.enter_context(nc.semaphore("sgp"))
        dgp = ctx.enter_context(nc.semaphore("dgp"))
        dout = ctx.enter_context(nc.semaphore("dout"))

        groups = [
            (g2, [("bx", 0, "b0"), ("bx", 2, "b2"),
                  ("bx", 1, "b1"), ("bx", 3, "b3")]),
            (g0, [("dl", 4, "d4"), ("bx", 4, "b4")]),
            (g1, [("dl", 0, "d0"), ("dl", 1, "d1"),
                  ("dl", 2, "d2"), ("dl", 3, "d3")]),
        ]
        srcs = {"bx": bx, "dl": dl}
        out_slots = ["d0", "b2", "d1", "b3", "d2", "u0", "a3", "u1"]

        with nc.Block() as block:

            @block.sync
            def _(sync):
                for c in range(NCH1):
                    for sem, planes in groups:
                        for (src, idx, dst) in planes:
                            sync.dma_start(
                                out=T[dst][:, c * FC1:(c + 1) * FC1],
                                in_=dchunk(srcs[src], idx, c),
                            ).then_inc(sem[c], 16)
                # late out-DMAs (ca/tx/ty) after every in-DMA is queued;
                # early planes go out through gpsimd's SWDGE ring
                ndma = 0
                for c in range(NCH1):
                    for (sem, thr, planes) in (
                            (sdve, 19 * c + 18, (4,)),         # ca
                            (sgp, 13 * c + 10, (6,)),          # tx
                            (sgp, 13 * c + 13, (7,))):         # ty
                        sync.wait_ge(sem, thr)
                        for pidx in planes:
                            sync.dma_start(
                                out=dchunk(out, pidx, c),
                                in_=T[out_slots[pidx]][:,
                                                       c * FC1:(c + 1) * FC1],
                            ).then_inc(dout, 16)
                            ndma += 1
                sync.wait_ge(dout, 16 * ndma)

            @block.scalar
            def _(scalar):
                def act(dst, src, func, bias=0.0, scale=1.0):
                    nc.scalar.activation(dst, src, func, bias=bias,
                                         scale=scale).then_inc(sact, 1)

                def phase_a(c):  # trig set: s2 -> sy, s4 -> a1
                    s = slice(c * FC1, (c + 1) * FC1)
                    scalar.wait_ge(g0[c], 32)
                    act(T["sy"][:, s], T["b4"][:, s], AF.Sin, scale=0.5)
                    act(T["a1"][:, s], T["b4"][:, s], AF.Sin, scale=0.25)

                def phase_b(c):  # natural_log_exp set
                    s = slice(c * FC1, (c + 1) * FC1)
                    scalar.wait_ge(g1[c], 64)
                    act(T["hwh"][:, s], T["d2"][:, s], AF.Exp,
                        bias=LN_HALF, scale=0.2)                        # +1
                    act(T["hhh"][:, s], T["d3"][:, s], AF.Exp,
                        bias=LN_HALF, scale=0.2)                        # +2
                    act(T["rs"][:, s], T["d4"][:, s], AF.Square)        # +3 q2
                    act(T["b4"][:, s], T["rs"][:, s], AF.Ln, bias=1.0)  # +4 lq
                    act(T["rs"][:, s], T["b4"][:, s], AF.Exp,
                        scale=-0.5)                                     # +5 rsq

                # per-chunk set alternation: 8 table loads, but ACT still
                # finishes well ahead of DVE and chunk 0 unblocks earliest
                for c in range(NCH1):
                    phase_a(c)
                    phase_b(c)

            A_END = [7 * c + 2 for c in range(NCH1)]   # after phase_a(c)
            B_HH = [7 * c + 4 for c in range(NCH1)]    # after hhh of phase_b(c)
            B_END = [7 * c + 7 for c in range(NCH1)]   # after rsq of phase_b(c)

            @block.vector
            def _(vector):
                for c in range(NCH1):
                    s = slice(c * FC1, (c + 1) * FC1)

                    def t(name):
                        return T[name][:, s]

                    def tt(dst, a, op, b):
                        nc.vector.tensor_tensor(
                            out=dst, in0=a, in1=b, op=op).then_inc(sdve, 1)

                    def stt(dst, a, scalar_, b):
                        nc.vector.scalar_tensor_tensor(
                            out=dst, in0=a, scalar=scalar_, in1=b,
                            op0=OP.mult, op1=OP.mult).then_inc(sdve, 1)

                    def ts2(dst, a, s1, s2_, op0, op1):
                        nc.vector.tensor_scalar(
                            out=dst, in0=a, scalar1=s1, scalar2=s2_,
                            op0=op0, op1=op1).then_inc(sdve, 1)

                    def ts1(dst, a, add):
                        nc.vector.tensor_scalar(
                            out=dst, in0=a, scalar1=add, scalar2=None,
                            op0=OP.add).then_inc(sdve, 1)

                    vector.wait_ge(g2[c], 64)
                    tt(t("w"), t("b2"), OP.subtract, t("b0"))            # 1 w
                    tt(t("h"), t("b3"), OP.subtract, t("b1"))            # 2 h
                    vector.wait_ge(g1[c], 64)
                    ts2(t("u0"), t("d0"), 0.1, 0.5, OP.mult, OP.add)     # 3 u0
                    tt(t("u0"), t("w"), OP.mult, t("u0"))                # 4 m
                    tt(t("b0"), t("u0"), OP.add, t("b0"))                # 5 pcx
                    ts2(t("u1"), t("d1"), 0.1, 0.5, OP.mult, OP.add)     # 6 u1
                    tt(t("u1"), t("h"), OP.mult, t("u1"))                # 7 m2
                    tt(t("b1"), t("u1"), OP.add, t("b1"))                # 8 pcy
                    # trig prep from s2 (sy), s4 (a1)
                    vector.wait_ge(sact, A_END[c])
                    stt(t("a2"), t("a1"), -2.0, t("a1"))                 # 9 q4
                    ts1(t("a2"), t("a2"), 1.0)                           # 10 c2
                    stt(t("a3"), t("sy"), 2.0, t("a2"))                  # 11 sA
                    stt(t("a1"), t("sy"), -2.0, t("sy"))                 # 12 qA
                    ts1(t("a1"), t("a1"), 1.0)                           # 13 cA
                    vector.wait_ge(sact, B_HH[c])
                    tt(t("w"), t("hwh"), OP.mult, t("w"))                # 14 hw
                    tt(t("h"), t("hhh"), OP.mult, t("h"))                # 15 hh
                    # ca chain (sa chain runs on gpsimd in parallel; keep
                    # d4/a1 read-only here -- gpsimd reads them concurrently)
                    vector.wait_ge(sact, B_END[c])
                    tt(t("d3"), t("d4"), OP.mult, t("a3"))               # 16 p2
                    tt(t("d3"), t("a1"), OP.subtract, t("d3"))           # 17 c'
                    tt(t("d2"), t("d3"), OP.mult, t("rs"))               # 18 ca
                    ts2(t("u1"), t("d2"), -1.0, 1.0, OP.mult, OP.add)    # 19 omc

            @block.gpsimd
            def _(gpsimd):
                for c in range(NCH1):
                    s = slice(c * FC1, (c + 1) * FC1)
                    base = 19 * c

                    def g(name):
                        return T[name][:, s]

                    def gtt(dst, a, op, b):
                        nc.gpsimd.tensor_tensor(
                            out=dst, in0=a, in1=b, op=op).then_inc(sgp, 1)

                    gpsimd.wait_ge(sdve, base + 15)
                    gtt(g("d0"), g("b0"), OP.subtract, g("w"))           # 1 xlo
                    gtt(g("b2"), g("b0"), OP.add, g("w"))                # 2 xhi
                    gtt(g("d1"), g("b1"), OP.subtract, g("h"))           # 3 ylo
                    gtt(g("b3"), g("b1"), OP.add, g("h"))                # 4 yhi
                    # sa chain: p = d4*cA, s' = p + sA, sa = s'*rsq
                    for pidx in (0, 1, 2, 3):
                        gpsimd.dma_start(
                            out=dchunk(out, pidx, c),
                            in_=T[out_slots[pidx]][:, c * FC1:(c + 1) * FC1],
                        ).then_inc(dgp, 16)
                    gpsimd.wait_ge(sact, B_END[c])
                    gtt(g("sy"), g("d4"), OP.mult, g("a1"))              # 5 p
                    gtt(g("sy"), g("sy"), OP.add, g("a3"))               # 6 s'
                    gtt(g("u0"), g("sy"), OP.mult, g("rs"))              # 7 sa
                    gpsimd.dma_start(
                        out=dchunk(out, 5, c),
                        in_=T[out_slots[5]][:, c * FC1:(c + 1) * FC1],
                    ).then_inc(dgp, 16)
                    gpsimd.wait_ge(sdve, base + 19)
                    gtt(g("a3"), g("b0"), OP.mult, g("u1"))              # 8 t1
                    gtt(g("d3"), g("u0"), OP.mult, g("b1"))              # 9 t2
                    gtt(g("a3"), g("a3"), OP.add, g("d3"))               # 10 tx
                    gtt(g("u1"), g("b1"), OP.mult, g("u1"))              # 11 t3
                    gtt(g("d3"), g("u0"), OP.mult, g("b0"))              # 12 t4
                    gtt(g("u1"), g("u1"), OP.subtract, g("d3"))          # 13 ty
                gpsimd.wait_ge(dgp, 16 * 5 * NCH1)

    return nc


def build_l2():
    nc = bass.Bass(detect_race_conditions=False)
    vin = nc.declare_dram_parameter("vin", [12, NR2], DT, isOutput=False)
    rotr = nc.declare_dram_parameter("rotr", [4, NR2], DT, isOutput=False)
    roto = nc.declare_dram_parameter("roto", [4, NO2], DT, isOutput=False)
    outr = nc.declare_dram_parameter("outr", [8, NR2], DT, isOutput=True)
    outo = nc.declare_dram_parameter("outo", [2, NO2], DT, isOutput=True)

    def dchunk(t, i, c, fc=FC2):
        return t[i].rearrange("(p f) -> p f", p=P)[:, c * fc:(c + 1) * fc]

    def dplane(t, i):
        return t[i].rearrange("(p f) -> p f", p=P)

    with ExitStack() as ctx:
        V = [ctx.enter_context(nc.sbuf_tensor(f"v{i}", [P, F2], DT))
             for i in range(12)]
        R = [ctx.enter_context(nc.sbuf_tensor(f"r{i}", [P, F2], DT))
             for i in range(4)]
        O = [ctx.enter_context(nc.sbuf_tensor(f"o{i}", [P, F2], DT))
             for i in range(8)]
        TA = ctx.enter_context(nc.sbuf_tensor("ta", [P, F2], DT))
        TB = ctx.enter_context(nc.sbuf_tensor("tb", [P, F2], DT))
        GA = ctx.enter_context(nc.sbuf_tensor("ga", [P, F2], DT))
        Q = [ctx.enter_context(nc.sbuf_tensor(f"q{i}", [P, F2O], DT))
             for i in range(4)]
        OX = ctx.enter_context(nc.sbuf_tensor("ox", [P, F2O], DT))
        OY = ctx.enter_context(nc.sbuf_tensor("oy", [P, F2O], DT))
        TC = ctx.enter_context(nc.sbuf_tensor("tc", [P, F2O], DT))
        dq = ctx.enter_context(nc.semaphore("dq"))
        dch = [ctx.enter_context(nc.semaphore(f"dch{c}")) for c in range(NCH2)]
        sdve = ctx.enter_context(nc.semaphore("sdve"))
        sgp = ctx.enter_context(nc.semaphore("sgp"))
        dout = ctx.enter_context(nc.semaphore("dout"))

        with nc.Block() as block:

            @block.sync
            def _(sync):
                for c in range(NCH2):
                    s = slice(c * FC2, (c + 1) * FC2)
                    for i in range(4):
                        sync.dma_start(out=R[i][:, s], in_=dchunk(rotr, i, c)
                                       ).then_inc(dch[c], 16)
                    for i in range(12):
                        sync.dma_start(out=V[i][:, s], in_=dchunk(vin, i, c)
                                       ).then_inc(dch[c], 16)
                for i in range(4):
                    sync.dma_start(out=Q[i][:], in_=dplane(roto, i)
                                   ).then_inc(dq, 16)
            @block.scalar
            def _(scalar):
                # ACT is idle in this kernel; use its HWDGE ring for the
                # out-DMAs so they don't queue behind the in-DMAs
                ndma = 0
                for c in range(NCH2):
                    for c4 in range(4):
                        scalar.wait_ge(sgp, 16 * c + 4 * c4 + 2)
                        nc.scalar.dma_start(
                            out=dchunk(outr, c4, c),
                            in_=O[c4][:, c * FC2:(c + 1) * FC2]
                        ).then_inc(dout, 16)
                        ndma += 1
                        scalar.wait_ge(sgp, 16 * c + 4 * c4 + 4)
                        nc.scalar.dma_start(
                            out=dchunk(outr, 4 + c4, c),
                            in_=O[4 + c4][:, c * FC2:(c + 1) * FC2]
                        ).then_inc(dout, 16)
                        ndma += 1
                scalar.wait_ge(sdve, 24 * NCH2 + 2)
                nc.scalar.dma_start(out=dplane(outo, 0), in_=OX[:]
                                    ).then_inc(dout, 16)
                ndma += 1
                scalar.wait_ge(sdve, 24 * NCH2 + 4)
                nc.scalar.dma_start(out=dplane(outo, 1), in_=OY[:]
                                    ).then_inc(dout, 16)
                ndma += 1
                scalar.wait_ge(dout, 16 * ndma)

            @block.vector
            def _(vector):
                def tt(dst, a, op, b):
                    nc.vector.tensor_tensor(out=dst, in0=a, in1=b,
                                            op=op).then_inc(sdve, 1)

                for c in range(NCH2):
                    s = slice(c * FC2, (c + 1) * FC2)
                    vector.wait_ge(dch[c], 256)
                    for c4 in range(4):
                        tt(TA[:, s], R[0][:, s], OP.mult, V[c4][:, s])
                        tt(TB[:, s], R[1][:, s], OP.mult, V[4 + c4][:, s])
                        tt(O[c4][:, s], TA[:, s], OP.subtract, TB[:, s])
                        tt(TA[:, s], R[1][:, s], OP.mult, V[c4][:, s])
                        tt(TB[:, s], R[0][:, s], OP.mult, V[4 + c4][:, s])
                        tt(O[4 + c4][:, s], TA[:, s], OP.add, TB[:, s])
                vector.wait_ge(dq, 64)
                tt(TC[:], Q[0][:], OP.subtract, Q[1][:])        # ca-sa
                tt(OX[:], TC[:], OP.add, Q[2][:])               # +tx
                tt(TC[:], Q[0][:], OP.add, Q[1][:])             # ca+sa
                tt(OY[:], TC[:], OP.add, Q[3][:])               # +ty

            @block.gpsimd
            def _(gpsimd):
                for c in range(NCH2):
                    s = slice(c * FC2, (c + 1) * FC2)
                    base = 24 * c
                    gpsimd.wait_ge(dch[c], 256)
                    for c4 in range(4):
                        nc.gpsimd.tensor_tensor(
                            out=GA[:, s], in0=R[2][:, s],
                            in1=V[8 + c4][:, s], op=OP.mult
                        ).then_inc(sgp, 1)                               # p3
                        gpsimd.wait_ge(sdve, base + 6 * c4 + 3)
                        nc.gpsimd.tensor_tensor(
                            out=O[c4][:, s], in0=O[c4][:, s],
                            in1=GA[:, s], op=OP.add).then_inc(sgp, 1)    # X
                        nc.gpsimd.tensor_tensor(
                            out=GA[:, s], in0=R[3][:, s],
                            in1=V[8 + c4][:, s], op=OP.mult
                        ).then_inc(sgp, 1)                               # q3
                        gpsimd.wait_ge(sdve, base + 6 * c4 + 6)
                        nc.gpsimd.tensor_tensor(
                            out=O[4 + c4][:, s], in0=O[4 + c4][:, s],
                            in1=GA[:, s], op=OP.add).then_inc(sgp, 1)    # Y

    return nc


# ---------------- host orchestration ----------------

_CACHE = {}


def _get_l1():
    if "l1" not in _CACHE:
        _CACHE["l1"] = build_l1()
    return _CACHE["l1"]


def _get_l2():
    if "l2" not in _CACHE:
        _CACHE["l2"] = build_l2()
    return _CACHE["l2"]


def _run(nc, in_maps, **kw):
    return run_bass_kernel_spmd(nc, in_maps, list(range(8)), **kw).results


def kernel(boxes, deltas):
    boxes = np.ascontiguousarray(np.asarray(boxes, dtype=np.float32))
    deltas = np.ascontiguousarray(np.asarray(deltas, dtype=np.float32))

    # ---- launch 1 ----
    in1 = []
    for b in range(B):
        bxp = np.zeros((5, NP1), np.float32)
        bxp[:, :N] = boxes[b].T
        dlp = np.zeros((5, NP1), np.float32)
        dlp[:, :N] = deltas[b].T
        in1.append({"bx": bxp, "dl": dlp})
    res1 = _run(_get_l1(), in1)
    planes = np.stack([res1[b]["out"][:, :N] for b in range(B)])  # [B, 8, N]

    # ---- host reshuffle ----
    comp = {0: 0, 1: 0, 2: 1, 3: 1, 4: 2, 5: 3, 6: 2, 7: 3}  # k -> plane idx
    Cflat = np.empty(96 * N, np.float32)
    for i in range(96):
        k, bsrc = divmod(i, 8)
        if k < 8:
            Cflat[i * N:(i + 1) * N] = planes[bsrc, comp[k]]
        else:
            Cflat[i * N:(i + 1) * N] = 1.0
    GR = planes[:, 4:8, :].transpose(1, 0, 2).reshape(4, B * N)  # ca,sa,tx,ty

    in2 = []
    for j in range(8):
        r0 = j * NRC
        r1 = min((j + 1) * NRC, NR)
        vin = np.zeros((12, NR2), np.float32)
        blk = Cflat[12 * r0: 12 * r0 + 12 * NR2]
        nv = len(blk) // 12
        vin[:, :nv] = blk[:12 * nv].reshape(nv, 12).T
        rotr = np.zeros((4, NR2), np.float32)
        rotr[:, :r1 - r0] = GR[:, r0:r1]
        o0 = NR + j * NOC
        o1 = min(NR + (j + 1) * NOC, 8 * N)
        roto = np.zeros((4, NO2), np.float32)
        roto[:, :o1 - o0] = GR[:, o0:o1]
        in2.append({"vin": vin, "rotr": rotr, "roto": roto})
    res2 = _run(_get_l2(), in2)

    OUT = np.empty((8 * N, 8), np.float32)
    for j in range(8):
        r0 = j * NRC
        r1 = min((j + 1) * NRC, NR)
        outr = res2[j]["outr"]
        OUT[r0:r1, 0::2] = outr[0:4, :r1 - r0].T
        OUT[r0:r1, 1::2] = outr[4:8, :r1 - r0].T
        o0 = NR + j * NOC
        o1 = min(NR + (j + 1) * NOC, 8 * N)
        outo = res2[j]["outo"]
        OUT[o0:o1, 0::2] = outo[0, :o1 - o0, None]
        OUT[o0:o1, 1::2] = outo[1, :o1 - o0, None]
    return OUT.reshape(B, N, 4, 2)

